# revision 30
# baseline (speedup 1.0000x reference)
"""CharRNN (128-layer stacked LSTM, H=64, T=128, B=1) on 8 Trainium2 cores.

Pipeline-parallel over layers (16 layers/core), wavefront ticks inside each
core. Per tick the 16 cells' gate matvecs run as 32 [128x128]x[128,1] fp16
matmuls accumulating onto a PSUM tile pre-filled with the biases (one
lhsT=bias/rhs=I16 matmul per group), so the scalar engine applies sigmoid
directly on PSUM. All four gates use sigmoid only (g rows are pre-scaled by
2 on the host; tanh(g) = 2*sigmoid(2g) - 1 is reconstructed with fused
scalar_tensor_tensor ops), which keeps the activation count at 2 per group.
Cells are split into two groups of 8 pipelined against each other so the
PE / Act / DVE / Pool engines overlap; copies and the t1/u products run on
the Pool queue (nc.gpsimd).
Boundary h-chunks move between cores with an AllGather every C timesteps;
per-core one-hot masks select the upstream slot so the SPMD program stays
address-uniform.
"""

import sys

sys.path.insert(0, "/opt/trn_rl_repo")

from contextlib import ExitStack

import numpy as np

import concourse.bass as bass
import concourse.mybir as mybir
from concourse import bacc, tile
from concourse.bass_utils import run_bass_kernel_spmd

F32 = mybir.dt.float32
HDT = mybir.dt.float16
HDT_NP = np.float16

H = 64
NL = 128
T = 128
V = 35
NCORE = 8
LPC = NL // NCORE          # 16 layers per core
G = 8                      # cells per pipeline group (2 groups)
C = 4                      # chunk of timesteps per pipeline round
L = 2                      # pipeline lag in rounds (handoff latency budget)
R = T // C                 # chunks
ROUNDS = R + L * (NCORE - 1)   # lockstep rounds

_CACHE = {}

SIG = mybir.ActivationFunctionType.Sigmoid
TANH = mybir.ActivationFunctionType.Tanh
MUL = mybir.AluOpType.mult
ADD = mybir.AluOpType.add
SUB = mybir.AluOpType.subtract


def _build():
    nc = bacc.Bacc()

    wts_d = nc.declare_dram_parameter("wts", [2 * LPC, 128, 128], HDT, isOutput=False)
    blh_d = nc.declare_dram_parameter("blh", [LPC, 2 * 128 * ROUNDS], HDT,
                                      isOutput=False)
    selm_d = nc.declare_dram_parameter("selm", [64, C * NCORE], HDT, isOutput=False)
    xest_d = nc.declare_dram_parameter("xest", [64, ROUNDS * C], HDT, isOutput=False)
    fmk_d = nc.declare_dram_parameter("fmk", [64, LPC * ROUNDS], HDT, isOutput=False)
    ident_d = nc.declare_dram_parameter("ident", [LPC, LPC], HDT, isOutput=False)
    ones_d = nc.declare_dram_parameter("onesv", [128, V], F32, isOutput=False)
    vct_d = nc.declare_dram_parameter("vct", [128, 1], F32, isOutput=False)
    wfc_d = nc.declare_dram_parameter("wfct", [64, V], HDT, isOutput=False)
    bfc_d = nc.declare_dram_parameter("bfc", [V, 1], F32, isOutput=False)
    iot_d = nc.declare_dram_parameter("iotar", [128, V], F32, isOutput=False)
    idn_d = nc.declare_dram_parameter("idn", [V, V], F32, isOutput=False)
    aginit_d = nc.declare_dram_parameter("aginit", [NCORE, 64, C], HDT,
                                         isOutput=False)
    out_d = nc.declare_dram_parameter("out_idx", [128, 1], F32, isOutput=True)

    ccin = [nc.dram_tensor(f"ccin{r}", [64, C], HDT) for r in range(ROUNDS)]
    agout = [
        nc.dram_tensor(f"agout{r}", [NCORE, 64, C], HDT, addr_space="Shared")
        for r in range(ROUNDS)
    ]
    hfin = nc.dram_tensor("hfin", [64, LPC], HDT)
    hfall = nc.dram_tensor("hfall", [NCORE, 64, LPC], HDT, addr_space="Shared")

    groups = [list(range(NCORE))]

    with tile.TileContext(nc) as tc, ExitStack() as ctx:
        const = ctx.enter_context(tc.tile_pool(name="const", bufs=1))
        state = ctx.enter_context(tc.tile_pool(name="state", bufs=1))
        work = ctx.enter_context(tc.tile_pool(name="work", bufs=3))
        gpool = ctx.enter_context(tc.tile_pool(name="gpool", bufs=2, space="PSUM"))

        # ---- constants ----
        wsb = const.tile([128, 2 * LPC * 128], HDT)
        nc.sync.dma_start(
            out=wsb.rearrange("k (j m) -> k j m", m=128),
            in_=wts_d[:].rearrange("j k m -> k j m"),
        )
        wsb_v = wsb.rearrange("k (j m) -> k j m", m=128)
        blh = const.tile([LPC, 2 * 128 * ROUNDS], HDT)  # per-round bias lhsT
        nc.sync.dma_start(out=blh[:], in_=blh_d[:])
        ident = const.tile([LPC, LPC], HDT)
        nc.sync.dma_start(out=ident[:], in_=ident_d[:])
        selm = const.tile([64, C * NCORE], HDT)
        nc.sync.dma_start(out=selm[:], in_=selm_d[:])
        xest = const.tile([64, ROUNDS * C], HDT)
        nc.sync.dma_start(out=xest[:], in_=xest_d[:])
        fmk = const.tile([64, LPC * ROUNDS], HDT)
        nc.sync.dma_start(out=fmk[:], in_=fmk_d[:])
        onesv = const.tile([128, V], F32)
        nc.sync.dma_start(out=onesv[:], in_=ones_d[:])
        vct = const.tile([128, 1], F32)
        nc.sync.dma_start(out=vct[:], in_=vct_d[:])
        wfct = const.tile([64, V], HDT)
        nc.sync.dma_start(out=wfct[:], in_=wfc_d[:])
        bfct = const.tile([V, 1], F32)
        nc.sync.dma_start(out=bfct[:], in_=bfc_d[:])
        iotar = const.tile([128, V], F32)
        nc.sync.dma_start(out=iotar[:], in_=iot_d[:])
        idn = const.tile([V, V], F32)
        nc.sync.dma_start(out=idn[:], in_=idn_d[:])

        # ---- persistent state ----
        vst = state.tile([128, LPC], HDT)   # rows 0:64 = y inputs, 64:128 = h
        cst = state.tile([64, LPC], F32)    # cell state
        hist = state.tile([64, LPC * ROUNDS], HDT)  # h snapshot per round
        nc.vector.memset(vst[:], 0.0)
        nc.vector.memset(cst[:], 0.0)
        nc.vector.memset(hist[:], 0.0)

        def emit_recv(r):
            # receive upstream chunk (written L rounds ago), one-hot select
            # slot k-1, add core0's x-embedding stream; emitted one round
            # early so the round boundary doesn't stall on this chain
            agprev = aginit_d if r < L else agout[r - L]
            slots = work.tile([64, NCORE * C], HDT, tag="slots", name="slots")
            nc.sync.dma_start(
                out=slots.rearrange("p (s t) -> p s t", t=C),
                in_=agprev[:].rearrange("s p t -> p s t"),
            )
            tmp2 = work.tile([64, C * NCORE], HDT, tag="tmp2", name="tmp2")
            nc.gpsimd.tensor_mul(
                tmp2.rearrange("p (t s) -> p t s", s=NCORE),
                slots.rearrange("p (s t) -> p t s", t=C),
                selm.rearrange("p (t s) -> p t s", s=NCORE),
            )
            rinch = work.tile([64, C], HDT, tag="inch", name="rinch")
            with nc.allow_low_precision(reason="one-hot slot select, fp16 exact"):
                nc.vector.tensor_reduce(
                    out=rinch[:],
                    in_=tmp2.rearrange("p (t s) -> p t s", s=NCORE),
                    axis=mybir.AxisListType.X,
                    op=ADD,
                )
            nc.gpsimd.tensor_add(rinch[:], rinch[:], xest[:, r * C:(r + 1) * C])
            return rinch

        inch = emit_recv(0)
        for r in range(ROUNDS):
            inch_next = None
            outch = work.tile([64, C], HDT, tag="outch")

            for t in range(C):
                pg0 = gpool.tile([128, 2 * G], F32, tag="g0", name="pg0")
                pg1 = gpool.tile([128, 2 * G], F32, tag="g1", name="pg1")
                sg0 = work.tile([128, 2 * G], F32, tag="s0", name="sg0")
                sg1 = work.tile([128, 2 * G], F32, tag="s1", name="sg1")
                pg = [pg0, pg1]
                sg = [sg0, sg1]

                # PE: bias prefill + 16 gate matmuls per group; the bias also
                # carries the -60 "inactive round" offsets on f/i/o rows so
                # sigmoid() forces c = h = 0 before this core's wavefront
                # arrives (replaces explicit state masking)
                for g in (0, 1):
                    bcol = (2 * r + g) * 128
                    nc.tensor.matmul(
                        pg[g][:], blh[:, bcol:bcol + 128], ident[:],
                        start=True, stop=False, skip_group_check=True,
                    )
                    for jj in range(G):
                        j = G * g + jj
                        nc.tensor.matmul(
                            pg[g][:, jj:jj + 1], wsb_v[:, 2 * j, :],
                            vst[:, j:j + 1],
                            start=False, stop=True, skip_group_check=True,
                        )
                        nc.tensor.matmul(
                            pg[g][:, G + jj:G + jj + 1], wsb_v[:, 2 * j + 1, :],
                            vst[:, j:j + 1],
                            start=False, stop=True, skip_group_check=True,
                        )

                # per group: sigmoid on psum, then fused elementwise chain
                t1 = [None, None]
                uu = [None, None]
                for g in (0, 1):
                    nc.scalar.activation(sg[g][:], pg[g][:], SIG)
                for g in (0, 1):
                    cs = cst[:, G * g:G * (g + 1)]
                    # t1 = sig(f) * c         (Pool, off the serial chain)
                    t1[g] = work.tile([64, G], F32, tag=f"t1{g}", name=f"t1{g}")
                    nc.gpsimd.tensor_mul(t1[g][:], sg[g][0:64, 0:G], cs)
                    # u = sig(i) * sig(2g)    (DVE, on chain)
                    uu[g] = work.tile([128, G], F32, tag=f"u{g}", name=f"u{g}")
                    nc.vector.tensor_mul(
                        uu[g][64:128, :], sg[g][64:128, 0:G], sg[g][64:128, G:2 * G])
                for g in (0, 1):
                    cs = cst[:, G * g:G * (g + 1)]
                    # s1 = 2*u - sig(i)       (DVE fused)
                    s1 = work.tile([64, G], F32, tag=f"s1{g}")
                    nc.vector.scalar_tensor_tensor(
                        s1[:], uu[g][64:128, :], 2.0, sg[g][64:128, 0:G],
                        op0=MUL, op1=SUB)
                    # c = s1 + t1             (DVE)
                    nc.vector.tensor_add(cs, s1[:], t1[g][:])
                    # tc = tanh(c)            (Act; tanh shares the sigmoid
                    # table set, so no table reload)
                    tc = work.tile([64, G], F32, tag=f"s2c{g}")
                    nc.scalar.activation(tc[:], cs, TANH)
                    # h = sig(o)*tanh(c) written twice on DVE: h columns and
                    # the shifted y columns for the next tick (no Pool hop
                    # before the next gate matmuls)
                    nc.vector.tensor_mul(
                        vst[64:128, G * g:G * (g + 1)],
                        sg[g][0:64, G:2 * G], tc[:])
                    if g == 0:
                        nc.vector.tensor_mul(
                            vst[0:64, 1:G + 1], sg[g][0:64, G:2 * G], tc[:])
                        nc.gpsimd.tensor_copy(
                            vst[0:64, 0:1], inch[:, t:t + 1])
                    else:
                        nc.vector.tensor_mul(
                            vst[0:64, G + 1:2 * G],
                            sg[g][0:64, G:2 * G - 1], tc[:, 0:G - 1])
                        nc.gpsimd.tensor_copy(
                            outch[:, t:t + 1], vst[64:128, LPC - 1:LPC])

                if t == C - 1 and r + 1 < ROUNDS:
                    inch_next = emit_recv(r + 1)

            # snapshot h (cols r, stride ROUNDS) and ship the boundary chunk
            # to the next core via two 1-hop pairwise AllGathers
            nc.gpsimd.tensor_copy(hist[:, r::ROUNDS], vst[64:128, :])
            nc.sync.dma_start(out=ccin[r][:], in_=outch[:])
            nc.gpsimd.collective_compute(
                "AllGather", mybir.AluOpType.bypass, replica_groups=groups,
                ins=[ccin[r][:]], outs=[agout[r][:]],
            )
            if inch_next is not None:
                inch = inch_next

        # ---- head: gather final h, logits, softmax over layers, argmax ----
        fsel = work.tile([64, LPC * ROUNDS], HDT, tag="fsel")
        nc.vector.tensor_mul(fsel[:], hist[:], fmk[:])
        fh = state.tile([64, LPC], HDT)
        with nc.allow_low_precision(reason="one-hot round select, exact in fp16"):
            nc.vector.tensor_reduce(
                out=fh[:], in_=fsel.rearrange("p (i r) -> p i r", r=ROUNDS),
                axis=mybir.AxisListType.X, op=mybir.AluOpType.add,
            )
        nc.sync.dma_start(out=hfin[:], in_=fh[:])
        nc.gpsimd.collective_compute(
            "AllGather", mybir.AluOpType.bypass, replica_groups=groups,
            ins=[hfin[:]], outs=[hfall[:]],
        )
        HT = state.tile([64, NL], HDT)
        nc.sync.dma_start(
            out=HT.rearrange("p (s i) -> p s i", i=LPC),
            in_=hfall[:].rearrange("s p i -> p s i"),
        )
        logp = gpool.tile([V, NL], F32, tag="logp", bufs=1)
        nc.tensor.matmul(logp[:], wfct[:], HT[:], start=True, stop=True)
        logits = work.tile([V, NL], F32, tag="logits")
        nc.scalar.add(logits[:], logp[:], bfct[:, 0:1])

        mx = work.tile([V, 1], F32, tag="mx")
        nc.vector.tensor_reduce(
            out=mx[:], in_=logits[:], axis=mybir.AxisListType.X, op=mybir.AluOpType.max)
        nmx = work.tile([V, 1], F32, tag="nmx")
        nc.scalar.mul(nmx[:], mx[:], -1.0)
        ex = work.tile([V, NL], F32, tag="ex")
        nc.scalar.activation(
            ex[:], logits[:], mybir.ActivationFunctionType.Exp, bias=nmx[:, 0:1])
        sm = work.tile([V, 1], F32, tag="sm")
        nc.vector.tensor_reduce(
            out=sm[:], in_=ex[:], axis=mybir.AxisListType.X, op=mybir.AluOpType.add)
        rsm = work.tile([V, 1], F32, tag="rsm")
        nc.vector.reciprocal(rsm[:], sm[:])
        probs = work.tile([V, NL], F32, tag="probs")
        nc.scalar.mul(probs[:], ex[:], rsm[:, 0:1])

        tp = gpool.tile([128, V], F32, tag="tp", bufs=1)
        nc.tensor.transpose(tp[:], probs[:], idn[:])
        m2 = work.tile([128, 1], F32, tag="m2")
        nc.vector.tensor_reduce(
            out=m2[:], in_=tp[:], axis=mybir.AxisListType.X, op=mybir.AluOpType.max)
        m2b = work.tile([128, V], F32, tag="m2b")
        nc.scalar.mul(m2b[:], onesv[:], m2[:, 0:1])
        eq = work.tile([128, V], F32, tag="eq")
        nc.vector.tensor_tensor(eq[:], tp[:], m2b[:], op=mybir.AluOpType.is_equal)
        val = work.tile([128, V], F32, tag="val")
        nc.vector.tensor_mul(val[:], eq[:], iotar[:])
        mr = work.tile([128, 1], F32, tag="mr")
        nc.vector.tensor_reduce(
            out=mr[:], in_=val[:], axis=mybir.AxisListType.X, op=mybir.AluOpType.max)
        idx = work.tile([128, 1], F32, tag="idx")
        nc.vector.tensor_sub(idx[:], vct[:], mr[:])
        nc.sync.dma_start(out=out_d[:], in_=idx[:])

    nc.finalize()
    return nc


def _prep_in_maps(inputs):
    x = np.asarray(inputs["x"]).astype(np.int64)
    embed = np.asarray(inputs["embed"], dtype=np.float32)
    xe = embed[x, 0]  # (T,)

    Wih_full = np.zeros((NL, 4 * H, H), np.float32)
    Wih_full[0, :, 0] = np.asarray(inputs["Wih0"], np.float32)[:, 0]
    Wih_full[1:] = np.asarray(inputs["Wih"], np.float32)
    Whh_full = np.concatenate(
        [np.asarray(inputs["Whh0"], np.float32)[None],
         np.asarray(inputs["Whh"], np.float32)], axis=0)
    b_full = np.concatenate(
        [(np.asarray(inputs["bih0"], np.float32)
          + np.asarray(inputs["bhh0"], np.float32))[None],
         np.asarray(inputs["bih"], np.float32)
         + np.asarray(inputs["bhh"], np.float32)], axis=0)  # (NL, 256)

    Wcat = np.concatenate([Wih_full, Whh_full], axis=2)      # (NL, 256, 128)
    # permute pytorch gate order [i f g o] -> [f i o g] so the device layout
    # has half-A rows = [f; i] and half-B rows = [o; g]; g rows are scaled by
    # 2 so tanh(g) = 2*sigmoid(2g) - 1 can be built from sigmoid outputs.
    perm = np.r_[64:128, 0:64, 192:256, 128:192]
    Wcat = Wcat[:, perm, :]
    b_full = b_full[:, perm]
    Wcat[:, 192:256, :] *= 2.0
    b_full = b_full.copy()
    b_full[:, 192:256] *= 2.0
    lhsT_all = np.ascontiguousarray(np.transpose(Wcat, (0, 2, 1)))  # (NL,128,256)

    wfct = np.asarray(inputs["Wfc"], np.float32).T.astype(HDT_NP)  # (64, V)
    bfc = np.asarray(inputs["bfc"], np.float32).reshape(V, 1)
    iotar = np.broadcast_to(
        (V - np.arange(V, dtype=np.float32))[None, :], (128, V)).copy()
    idn = np.eye(V, dtype=np.float32)
    ident = np.eye(LPC, dtype=HDT_NP)
    aginit = np.zeros((NCORE, 64, C), HDT_NP)

    in_maps = []
    for k in range(NCORE):
        lhsT_k = lhsT_all[k * LPC:(k + 1) * LPC]  # (LPC, 128, 256)
        wts = (lhsT_k.reshape(LPC, 128, 2, 128)
               .transpose(0, 2, 1, 3)
               .reshape(2 * LPC, 128, 128).astype(HDT_NP))
        bk = b_full[k * LPC:(k + 1) * LPC]  # (LPC, 256) rows=cells, [A|B]
        # bias-matmul lhsT: row n of group-g slice = bias vector of psum col n
        bbase = np.zeros((LPC, 2, 128), np.float32)
        for g in (0, 1):
            for n in range(2 * G):
                cell = G * g + (n % G)
                half = 0 if n < G else 1
                bbase[n, g, :] = bk[cell, 128 * half:128 * (half + 1)]
        # inactive-round variant: -60 on f/i/o rows so sigmoid -> 0
        binact = bbase.copy()
        binact[0:G, :, :] -= 60.0            # A block cols: f (0:64) & i (64:128)
        binact[G:2 * G, :, 0:64] -= 60.0     # B block cols: o rows only
        blh = np.zeros((LPC, ROUNDS, 2, 128), HDT_NP)
        for r in range(ROUNDS):
            blh[:, r] = (bbase if r >= L * k else binact).astype(HDT_NP)

        selm = np.zeros((64, C, NCORE), HDT_NP)
        if k > 0:
            selm[:, :, k - 1] = 1.0
        xest = np.zeros((64, ROUNDS * C), HDT_NP)
        if k == 0:
            xest[0, :T] = xe.astype(HDT_NP)
        fmk = np.zeros((64, LPC, ROUNDS), HDT_NP)
        fmk[:, :, L * k + R - 1] = 1.0

        in_maps.append({
            "wts": wts,
            "blh": blh.reshape(LPC, ROUNDS * 2 * 128),
            "selm": selm.reshape(64, C * NCORE),
            "xest": xest,
            "fmk": fmk.reshape(64, LPC * ROUNDS),
            "ident": ident,
            "onesv": np.ones((128, V), np.float32),
            "vct": np.full((128, 1), float(V), np.float32),
            "wfct": wfct,
            "bfc": bfc,
            "iotar": iotar,
            "idn": idn,
            "aginit": aginit,
        })
    return in_maps


def _run(inputs, trace=False):
    if "nc" not in _CACHE:
        _CACHE["nc"] = _build()
    nc = _CACHE["nc"]
    in_maps = _prep_in_maps(inputs)
    res = run_bass_kernel_spmd(nc, in_maps, list(range(NCORE)), trace=trace)
    out = np.asarray(res.results[0]["out_idx"], np.float32).reshape(NL)
    idx = np.rint(out).astype(np.int32)
    return idx, res


def kernel(**inputs) -> np.ndarray:
    idx, _ = _run(inputs, trace=False)
    return idx


# revision 31
# speedup vs baseline: 1.0550x; 1.0550x over previous
"""CharRNN (128-layer stacked LSTM, H=64, T=128, B=1) on 8 Trainium2 cores.

Pipeline-parallel over layers (16 layers/core), wavefront ticks inside each
core. Per tick the 16 cells' gate matvecs run as 32 [128x128]x[128,1] fp16
matmuls accumulating onto a PSUM tile pre-filled with the biases (one
lhsT=bias/rhs=I16 matmul per group), so the scalar engine applies sigmoid
directly on PSUM. All four gates use sigmoid only (g rows are pre-scaled by
2 on the host; tanh(g) = 2*sigmoid(2g) - 1 is reconstructed with fused
scalar_tensor_tensor ops), which keeps the activation count at 2 per group.
Cells are split into two groups of 8 pipelined against each other so the
PE / Act / DVE / Pool engines overlap; copies and the t1/u products run on
the Pool queue (nc.gpsimd).
Boundary h-chunks move between cores with an AllGather every C timesteps;
per-core one-hot masks select the upstream slot so the SPMD program stays
address-uniform.
"""

import sys

sys.path.insert(0, "/opt/trn_rl_repo")

from contextlib import ExitStack

import numpy as np

import concourse.bass as bass
import concourse.mybir as mybir
from concourse import bacc, tile
from concourse.bass_utils import run_bass_kernel_spmd

F32 = mybir.dt.float32
HDT = mybir.dt.float16
HDT_NP = np.float16

H = 64
NL = 128
T = 128
V = 35
NCORE = 8
LPC = NL // NCORE          # 16 layers per core
G = 8                      # cells per pipeline group (2 groups)
C = 4                      # chunk of timesteps per pipeline round
L = 2                      # pipeline lag in rounds (handoff latency budget)
R = T // C                 # chunks
ROUNDS = R + L * (NCORE - 1)   # lockstep rounds

_CACHE = {}

SIG = mybir.ActivationFunctionType.Sigmoid
TANH = mybir.ActivationFunctionType.Tanh
MUL = mybir.AluOpType.mult
ADD = mybir.AluOpType.add
SUB = mybir.AluOpType.subtract


def _build():
    nc = bacc.Bacc()

    wts_d = nc.declare_dram_parameter("wts", [2 * LPC, 128, 128], HDT, isOutput=False)
    blh_d = nc.declare_dram_parameter("blh", [LPC, 2 * 128 * ROUNDS], HDT,
                                      isOutput=False)
    selm_d = nc.declare_dram_parameter("selm", [64, C * NCORE], HDT, isOutput=False)
    xest_d = nc.declare_dram_parameter("xest", [64, ROUNDS * C], HDT, isOutput=False)
    fmk_d = nc.declare_dram_parameter("fmk", [64, LPC * ROUNDS], HDT, isOutput=False)
    ident_d = nc.declare_dram_parameter("ident", [LPC, LPC], HDT, isOutput=False)
    ones_d = nc.declare_dram_parameter("onesv", [128, V], F32, isOutput=False)
    vct_d = nc.declare_dram_parameter("vct", [128, 1], F32, isOutput=False)
    wfc_d = nc.declare_dram_parameter("wfct", [64, V], HDT, isOutput=False)
    bfc_d = nc.declare_dram_parameter("bfc", [V, 1], F32, isOutput=False)
    iot_d = nc.declare_dram_parameter("iotar", [128, V], F32, isOutput=False)
    idn_d = nc.declare_dram_parameter("idn", [V, V], F32, isOutput=False)
    aginit_d = nc.declare_dram_parameter("aginit", [NCORE, 64, C], HDT,
                                         isOutput=False)
    out_d = nc.declare_dram_parameter("out_idx", [128, 1], F32, isOutput=True)

    ccin = [nc.dram_tensor(f"ccin{r}", [64, C], HDT) for r in range(ROUNDS)]
    agout = [
        nc.dram_tensor(f"agout{r}", [NCORE, 64, C], HDT, addr_space="Shared")
        for r in range(ROUNDS)
    ]
    hfin = nc.dram_tensor("hfin", [64, LPC], HDT)
    hfall = nc.dram_tensor("hfall", [NCORE, 64, LPC], HDT, addr_space="Shared")

    groups = [list(range(NCORE))]

    with tile.TileContext(nc) as tc, ExitStack() as ctx:
        const = ctx.enter_context(tc.tile_pool(name="const", bufs=1))
        state = ctx.enter_context(tc.tile_pool(name="state", bufs=1))
        work = ctx.enter_context(tc.tile_pool(name="work", bufs=3))
        gpool = ctx.enter_context(tc.tile_pool(name="gpool", bufs=2, space="PSUM"))

        # ---- constants ----
        wsb = const.tile([128, 2 * LPC * 128], HDT)
        nc.sync.dma_start(
            out=wsb.rearrange("k (j m) -> k j m", m=128),
            in_=wts_d[:].rearrange("j k m -> k j m"),
        )
        wsb_v = wsb.rearrange("k (j m) -> k j m", m=128)
        blh = const.tile([LPC, 2 * 128 * ROUNDS], HDT)  # per-round bias lhsT
        nc.sync.dma_start(out=blh[:], in_=blh_d[:])
        ident = const.tile([LPC, LPC], HDT)
        nc.sync.dma_start(out=ident[:], in_=ident_d[:])
        selm = const.tile([64, C * NCORE], HDT)
        nc.sync.dma_start(out=selm[:], in_=selm_d[:])
        xest = const.tile([64, ROUNDS * C], HDT)
        nc.sync.dma_start(out=xest[:], in_=xest_d[:])
        fmk = const.tile([64, LPC * ROUNDS], HDT)
        nc.sync.dma_start(out=fmk[:], in_=fmk_d[:])
        onesv = const.tile([128, V], F32)
        nc.sync.dma_start(out=onesv[:], in_=ones_d[:])
        vct = const.tile([128, 1], F32)
        nc.sync.dma_start(out=vct[:], in_=vct_d[:])
        wfct = const.tile([64, V], HDT)
        nc.sync.dma_start(out=wfct[:], in_=wfc_d[:])
        bfct = const.tile([V, 1], F32)
        nc.sync.dma_start(out=bfct[:], in_=bfc_d[:])
        iotar = const.tile([128, V], F32)
        nc.sync.dma_start(out=iotar[:], in_=iot_d[:])
        idn = const.tile([V, V], F32)
        nc.sync.dma_start(out=idn[:], in_=idn_d[:])

        # ---- persistent state ----
        vst = state.tile([128, LPC], HDT)   # rows 0:64 = y inputs, 64:128 = h
        cst = state.tile([64, LPC], F32)    # cell state
        hist = state.tile([64, LPC * ROUNDS], HDT)  # h snapshot per round
        nc.vector.memset(vst[:], 0.0)
        nc.vector.memset(cst[:], 0.0)
        nc.vector.memset(hist[:], 0.0)

        def emit_recv(r):
            # receive upstream chunk (written L rounds ago), one-hot select
            # slot k-1, add core0's x-embedding stream; emitted one round
            # early so the round boundary doesn't stall on this chain
            agprev = aginit_d if r < L else agout[r - L]
            slots = work.tile([64, NCORE * C], HDT, tag="slots", name="slots")
            nc.sync.dma_start(
                out=slots.rearrange("p (s t) -> p s t", t=C),
                in_=agprev[:].rearrange("s p t -> p s t"),
            )
            tmp2 = work.tile([64, C * NCORE], HDT, tag="tmp2", name="tmp2")
            nc.gpsimd.tensor_mul(
                tmp2.rearrange("p (t s) -> p t s", s=NCORE),
                slots.rearrange("p (s t) -> p t s", t=C),
                selm.rearrange("p (t s) -> p t s", s=NCORE),
            )
            rinch = work.tile([64, C], HDT, tag="inch", name="rinch")
            with nc.allow_low_precision(reason="one-hot slot select, fp16 exact"):
                nc.vector.tensor_reduce(
                    out=rinch[:],
                    in_=tmp2.rearrange("p (t s) -> p t s", s=NCORE),
                    axis=mybir.AxisListType.X,
                    op=ADD,
                )
            nc.gpsimd.tensor_add(rinch[:], rinch[:], xest[:, r * C:(r + 1) * C])
            return rinch

        inch = emit_recv(0)
        for r in range(ROUNDS):
            inch_next = None
            outch = work.tile([64, C], HDT, tag="outch")

            for t in range(C):
                pg0 = gpool.tile([128, 2 * G], F32, tag="g0", name="pg0")
                pg1 = gpool.tile([128, 2 * G], F32, tag="g1", name="pg1")
                sg0 = work.tile([128, 2 * G], F32, tag="s0", name="sg0")
                sg1 = work.tile([128, 2 * G], F32, tag="s1", name="sg1")
                pg = [pg0, pg1]
                sg = [sg0, sg1]

                # PE: bias prefill + 16 gate matmuls per group; the bias also
                # carries the -60 "inactive round" offsets on f/i/o rows so
                # sigmoid() forces c = h = 0 before this core's wavefront
                # arrives (replaces explicit state masking)
                for g in (0, 1):
                    bcol = (2 * r + g) * 128
                    nc.tensor.matmul(
                        pg[g][:], blh[:, bcol:bcol + 128], ident[:],
                        start=True, stop=False, skip_group_check=True,
                    )
                    for jj in range(G):
                        j = G * g + jj
                        nc.tensor.matmul(
                            pg[g][:, jj:jj + 1], wsb_v[:, 2 * j, :],
                            vst[:, j:j + 1],
                            start=False, stop=True, skip_group_check=True,
                        )
                        nc.tensor.matmul(
                            pg[g][:, G + jj:G + jj + 1], wsb_v[:, 2 * j + 1, :],
                            vst[:, j:j + 1],
                            start=False, stop=True, skip_group_check=True,
                        )

                # per group: sigmoid on psum, then fused elementwise chain
                for g in (0, 1):
                    nc.scalar.activation(sg[g][:], pg[g][:], SIG)
                # emit each group's c-chain contiguously so group B's ops
                # don't sit ahead of c_A in the in-order DVE queue
                tcs = [None, None]
                for g in (0, 1):
                    cs = cst[:, G * g:G * (g + 1)]
                    # t1 = sig(f) * c         (Pool, off the serial chain)
                    t1 = work.tile([64, G], F32, tag=f"t1{g}", name=f"t1{g}")
                    nc.gpsimd.tensor_mul(t1[:], sg[g][0:64, 0:G], cs)
                    # u = sig(i) * sig(2g)    (DVE, on chain)
                    uu = work.tile([128, G], F32, tag=f"u{g}", name=f"u{g}")
                    nc.vector.tensor_mul(
                        uu[64:128, :], sg[g][64:128, 0:G], sg[g][64:128, G:2 * G])
                    # s1 = 2*u - sig(i)       (DVE fused)
                    s1 = work.tile([64, G], F32, tag=f"s1{g}")
                    nc.vector.scalar_tensor_tensor(
                        s1[:], uu[64:128, :], 2.0, sg[g][64:128, 0:G],
                        op0=MUL, op1=SUB)
                    # c = s1 + t1             (DVE)
                    nc.vector.tensor_add(cs, s1[:], t1[:])
                    # tc = tanh(c)            (Act; tanh shares the sigmoid
                    # table set, so no table reload)
                    tcs[g] = work.tile([64, G], F32, tag=f"s2c{g}",
                                       name=f"tc{g}")
                    nc.scalar.activation(tcs[g][:], cs, TANH)
                for g in (0, 1):
                    tc = tcs[g]
                    # h = sig(o)*tanh(c) written twice on DVE: h columns and
                    # the shifted y columns for the next tick (no Pool hop
                    # before the next gate matmuls)
                    nc.vector.tensor_mul(
                        vst[64:128, G * g:G * (g + 1)],
                        sg[g][0:64, G:2 * G], tc[:])
                    if g == 0:
                        nc.vector.tensor_mul(
                            vst[0:64, 1:G + 1], sg[g][0:64, G:2 * G], tc[:])
                        nc.gpsimd.tensor_copy(
                            vst[0:64, 0:1], inch[:, t:t + 1])
                    else:
                        nc.vector.tensor_mul(
                            vst[0:64, G + 1:2 * G],
                            sg[g][0:64, G:2 * G - 1], tc[:, 0:G - 1])
                        nc.gpsimd.tensor_copy(
                            outch[:, t:t + 1], vst[64:128, LPC - 1:LPC])

                if t == C - 1 and r + 1 < ROUNDS:
                    inch_next = emit_recv(r + 1)

            # snapshot h (cols r, stride ROUNDS) and ship the boundary chunk
            # to the next core via two 1-hop pairwise AllGathers
            nc.gpsimd.tensor_copy(hist[:, r::ROUNDS], vst[64:128, :])
            nc.sync.dma_start(out=ccin[r][:], in_=outch[:])
            nc.gpsimd.collective_compute(
                "AllGather", mybir.AluOpType.bypass, replica_groups=groups,
                ins=[ccin[r][:]], outs=[agout[r][:]],
            )
            if inch_next is not None:
                inch = inch_next

        # ---- head: gather final h, logits, softmax over layers, argmax ----
        fsel = work.tile([64, LPC * ROUNDS], HDT, tag="fsel")
        nc.vector.tensor_mul(fsel[:], hist[:], fmk[:])
        fh = state.tile([64, LPC], HDT)
        with nc.allow_low_precision(reason="one-hot round select, exact in fp16"):
            nc.vector.tensor_reduce(
                out=fh[:], in_=fsel.rearrange("p (i r) -> p i r", r=ROUNDS),
                axis=mybir.AxisListType.X, op=mybir.AluOpType.add,
            )
        nc.sync.dma_start(out=hfin[:], in_=fh[:])
        nc.gpsimd.collective_compute(
            "AllGather", mybir.AluOpType.bypass, replica_groups=groups,
            ins=[hfin[:]], outs=[hfall[:]],
        )
        HT = state.tile([64, NL], HDT)
        nc.sync.dma_start(
            out=HT.rearrange("p (s i) -> p s i", i=LPC),
            in_=hfall[:].rearrange("s p i -> p s i"),
        )
        logp = gpool.tile([V, NL], F32, tag="logp", bufs=1)
        nc.tensor.matmul(logp[:], wfct[:], HT[:], start=True, stop=True)
        logits = work.tile([V, NL], F32, tag="logits")
        nc.scalar.add(logits[:], logp[:], bfct[:, 0:1])

        mx = work.tile([V, 1], F32, tag="mx")
        nc.vector.tensor_reduce(
            out=mx[:], in_=logits[:], axis=mybir.AxisListType.X, op=mybir.AluOpType.max)
        nmx = work.tile([V, 1], F32, tag="nmx")
        nc.scalar.mul(nmx[:], mx[:], -1.0)
        ex = work.tile([V, NL], F32, tag="ex")
        nc.scalar.activation(
            ex[:], logits[:], mybir.ActivationFunctionType.Exp, bias=nmx[:, 0:1])
        sm = work.tile([V, 1], F32, tag="sm")
        nc.vector.tensor_reduce(
            out=sm[:], in_=ex[:], axis=mybir.AxisListType.X, op=mybir.AluOpType.add)
        rsm = work.tile([V, 1], F32, tag="rsm")
        nc.vector.reciprocal(rsm[:], sm[:])
        probs = work.tile([V, NL], F32, tag="probs")
        nc.scalar.mul(probs[:], ex[:], rsm[:, 0:1])

        tp = gpool.tile([128, V], F32, tag="tp", bufs=1)
        nc.tensor.transpose(tp[:], probs[:], idn[:])
        m2 = work.tile([128, 1], F32, tag="m2")
        nc.vector.tensor_reduce(
            out=m2[:], in_=tp[:], axis=mybir.AxisListType.X, op=mybir.AluOpType.max)
        m2b = work.tile([128, V], F32, tag="m2b")
        nc.scalar.mul(m2b[:], onesv[:], m2[:, 0:1])
        eq = work.tile([128, V], F32, tag="eq")
        nc.vector.tensor_tensor(eq[:], tp[:], m2b[:], op=mybir.AluOpType.is_equal)
        val = work.tile([128, V], F32, tag="val")
        nc.vector.tensor_mul(val[:], eq[:], iotar[:])
        mr = work.tile([128, 1], F32, tag="mr")
        nc.vector.tensor_reduce(
            out=mr[:], in_=val[:], axis=mybir.AxisListType.X, op=mybir.AluOpType.max)
        idx = work.tile([128, 1], F32, tag="idx")
        nc.vector.tensor_sub(idx[:], vct[:], mr[:])
        nc.sync.dma_start(out=out_d[:], in_=idx[:])

    nc.finalize()
    return nc


def _prep_in_maps(inputs):
    x = np.asarray(inputs["x"]).astype(np.int64)
    embed = np.asarray(inputs["embed"], dtype=np.float32)
    xe = embed[x, 0]  # (T,)

    Wih_full = np.zeros((NL, 4 * H, H), np.float32)
    Wih_full[0, :, 0] = np.asarray(inputs["Wih0"], np.float32)[:, 0]
    Wih_full[1:] = np.asarray(inputs["Wih"], np.float32)
    Whh_full = np.concatenate(
        [np.asarray(inputs["Whh0"], np.float32)[None],
         np.asarray(inputs["Whh"], np.float32)], axis=0)
    b_full = np.concatenate(
        [(np.asarray(inputs["bih0"], np.float32)
          + np.asarray(inputs["bhh0"], np.float32))[None],
         np.asarray(inputs["bih"], np.float32)
         + np.asarray(inputs["bhh"], np.float32)], axis=0)  # (NL, 256)

    Wcat = np.concatenate([Wih_full, Whh_full], axis=2)      # (NL, 256, 128)
    # permute pytorch gate order [i f g o] -> [f i o g] so the device layout
    # has half-A rows = [f; i] and half-B rows = [o; g]; g rows are scaled by
    # 2 so tanh(g) = 2*sigmoid(2g) - 1 can be built from sigmoid outputs.
    perm = np.r_[64:128, 0:64, 192:256, 128:192]
    Wcat = Wcat[:, perm, :]
    b_full = b_full[:, perm]
    Wcat[:, 192:256, :] *= 2.0
    b_full = b_full.copy()
    b_full[:, 192:256] *= 2.0
    lhsT_all = np.ascontiguousarray(np.transpose(Wcat, (0, 2, 1)))  # (NL,128,256)

    wfct = np.asarray(inputs["Wfc"], np.float32).T.astype(HDT_NP)  # (64, V)
    bfc = np.asarray(inputs["bfc"], np.float32).reshape(V, 1)
    iotar = np.broadcast_to(
        (V - np.arange(V, dtype=np.float32))[None, :], (128, V)).copy()
    idn = np.eye(V, dtype=np.float32)
    ident = np.eye(LPC, dtype=HDT_NP)
    aginit = np.zeros((NCORE, 64, C), HDT_NP)

    in_maps = []
    for k in range(NCORE):
        lhsT_k = lhsT_all[k * LPC:(k + 1) * LPC]  # (LPC, 128, 256)
        wts = (lhsT_k.reshape(LPC, 128, 2, 128)
               .transpose(0, 2, 1, 3)
               .reshape(2 * LPC, 128, 128).astype(HDT_NP))
        bk = b_full[k * LPC:(k + 1) * LPC]  # (LPC, 256) rows=cells, [A|B]
        # bias-matmul lhsT: row n of group-g slice = bias vector of psum col n
        bbase = np.zeros((LPC, 2, 128), np.float32)
        for g in (0, 1):
            for n in range(2 * G):
                cell = G * g + (n % G)
                half = 0 if n < G else 1
                bbase[n, g, :] = bk[cell, 128 * half:128 * (half + 1)]
        # inactive-round variant: -60 on f/i/o rows so sigmoid -> 0
        binact = bbase.copy()
        binact[0:G, :, :] -= 60.0            # A block cols: f (0:64) & i (64:128)
        binact[G:2 * G, :, 0:64] -= 60.0     # B block cols: o rows only
        blh = np.zeros((LPC, ROUNDS, 2, 128), HDT_NP)
        for r in range(ROUNDS):
            blh[:, r] = (bbase if r >= L * k else binact).astype(HDT_NP)

        selm = np.zeros((64, C, NCORE), HDT_NP)
        if k > 0:
            selm[:, :, k - 1] = 1.0
        xest = np.zeros((64, ROUNDS * C), HDT_NP)
        if k == 0:
            xest[0, :T] = xe.astype(HDT_NP)
        fmk = np.zeros((64, LPC, ROUNDS), HDT_NP)
        fmk[:, :, L * k + R - 1] = 1.0

        in_maps.append({
            "wts": wts,
            "blh": blh.reshape(LPC, ROUNDS * 2 * 128),
            "selm": selm.reshape(64, C * NCORE),
            "xest": xest,
            "fmk": fmk.reshape(64, LPC * ROUNDS),
            "ident": ident,
            "onesv": np.ones((128, V), np.float32),
            "vct": np.full((128, 1), float(V), np.float32),
            "wfct": wfct,
            "bfc": bfc,
            "iotar": iotar,
            "idn": idn,
            "aginit": aginit,
        })
    return in_maps


def _run(inputs, trace=False):
    if "nc" not in _CACHE:
        _CACHE["nc"] = _build()
    nc = _CACHE["nc"]
    in_maps = _prep_in_maps(inputs)
    res = run_bass_kernel_spmd(nc, in_maps, list(range(NCORE)), trace=trace)
    out = np.asarray(res.results[0]["out_idx"], np.float32).reshape(NL)
    idx = np.rint(out).astype(np.int32)
    return idx, res


def kernel(**inputs) -> np.ndarray:
    idx, _ = _run(inputs, trace=False)
    return idx
